# revision 13
# baseline (speedup 1.0000x reference)
"""Trainium2 Bass kernel for a top-2 MoE layer (8 experts), expert-parallel
across 8 NeuronCores, with a mixed-precision schedule.

Math (per reference):
    logits = x @ router_w                    # [S, E] fp32
    top2 vals/idx; gates = softmax(top2)     # [S, 2]
    out = sum_e gate_e * (silu(x@w1[e]) * (x@w3[e])) @ w2[e]

Distribution strategy (expert-parallel, host-side dispatch): the router GEMM
is 0.05% of total FLOPs, so the host computes it exactly in fp32 and
dispatches (token, expert) pairs to the 8 cores. Each core's program is a
pure streaming SwiGLU FFN over a fixed schedule of 4 bf16 weight "segments"
(60+58+55+52 = 225 units of 16 tokens = 3600 bf16 token slots per core) plus
one fp8 segment of 512 tokens; the host bin-packs each expert's bf16 token
list into the 24 bf16 (core, segment) slots, while the fp8 slot on core e
always runs expert e.

Mixed precision: the output tolerance (2e-2) far exceeds bf16 error
(4.4e-3), and each (token, expert) pair's error contribution scales with
its gate weight. The 512 lowest-gate second-rank pairs of each expert run
entirely in fp8-e4m3 through the PE's DoubleRow mode (2 k-tiles per matmul
= 2x bf16 FLOP throughput); per-pair fp8 relative error is ~6.6e-2 and the
selected pairs carry ~5% of output energy, so the end-to-end error lands at
~1.6e-2 (measured). fp8 operands use global power-of-two scales (Sx=16,
Sw=256, Sh=8) folded into activation-scale immediates on the device; fp8
tiles reuse the bf16 tile-pool tags (half the bytes), so SBUF usage is
unchanged.

The device pipeline per block (up to 512 tokens): DMA x-block -> 22x
(8 matmuls w1 + 8 matmuls w3 -> PSUM; Silu on Scalar; mult on Vector ->
s_all) -> GEMM2 (8 output tiles x 22 matmuls, w2 streamed) -> DMA out.
GEMM2 for block b is emitted after GEMM1/3 of block b+1 within a segment so
the PE never waits on the Vector engine; it is flushed at segment end so the
next segment's w1/w3 DMAs overlap the last two GEMM2s. Gates are applied on
the host during the final gather-combine (y is linear in w2's output, so
the device returns ungated per-pair outputs in a transposed [d, token]
layout and the host does out[t] = g0*y[pos0[t]] + g1*y[pos1[t]]).
"""

import os
import sys

for _p in ("/opt/trn_rl_repo",):
    if _p not in sys.path and os.path.isdir(_p):
        sys.path.insert(0, _p)

from contextlib import ExitStack
from dataclasses import dataclass

import numpy as np
import ml_dtypes

from concourse import bacc, bass, mybir
import concourse.tile as tile

F32 = mybir.dt.float32
BF16 = mybir.dt.bfloat16
F8 = mybir.dt.float8e4
DRMODE = mybir.MatmulPerfMode.DoubleRow
E4NP = ml_dtypes.float8_e4m3
UNIT = 16  # dispatch granularity in tokens

# global fp8 scales (power of two; immediates baked into the program)
SX, SW, SH = 16.0, 256.0, 8.0
INV1 = 1.0 / (SX * SW)
C3 = SH / (SX * SW)
INV2 = 1.0 / (SH * SW)


@dataclass(frozen=True)
class Cfg:
    S: int = 16384      # tokens
    D: int = 1024       # d_model
    H: int = 2816       # hidden
    E: int = 8          # experts == n_cores
    SEG_UNITS: tuple = (50, 53, 56, 58)  # 16-token units per bf16 segment
    FP8_TOK: int = 640  # fp8 token slots per core (expert = core)

    @property
    def DC(self):
        return self.D // 128

    @property
    def HC(self):
        return self.H // 128

    @property
    def BF16_TOK(self):
        return UNIT * sum(self.SEG_UNITS)

    @property
    def TPC(self):
        return self.BF16_TOK + self.FP8_TOK  # token slots per core


REAL = Cfg()


BMAX = 512 // UNIT  # max units per matmul block (PSUM bank = 512 fp32)


def _even_split(n, cap):
    nb = -(-n // cap)
    base, rem = divmod(n, nb)
    return [base + (1 if i < rem else 0) for i in range(nb)]


def _blocks_of(nunits):
    """Split a segment of `nunits` UNIT-token units into matmul blocks of
    at most 512 tokens, smallest first. The LAST block of a segment is the
    flush GEMM2 that hides the next segment's w13 reload (11.5MB ~ 35us),
    so it should be a full 512-token block (37.5us of PE work); blocks
    under ~290 tokens also lose matmul efficiency."""
    if nunits <= BMAX:
        return [UNIT * nunits]
    split = [BMAX] + _even_split(nunits - BMAX, BMAX)
    if min(split) * UNIT < 288:
        split = _even_split(nunits, BMAX)
    return [UNIT * s for s in sorted(split)]


def build_program(cfg: Cfg, debug: bool = False):
    c = cfg
    NSEG = len(c.SEG_UNITS)
    seg_blocks = [_blocks_of(u) for u in c.SEG_UNITS]

    nc = bacc.Bacc(
        "TRN2", target_bir_lowering=False, debug=debug, num_devices=c.E
    )

    xin = nc.dram_tensor(
        "xin", [128, c.DC * c.BF16_TOK], BF16, kind="ExternalInput"
    ).ap()
    xin8 = nc.dram_tensor(
        "xin8", [128, c.DC * c.FP8_TOK], F8, kind="ExternalInput"
    ).ap()
    w13_d = [
        nc.dram_tensor(
            f"w13_s{i}", [128, c.HC * 2 * c.DC * 128], BF16, kind="ExternalInput"
        ).ap()
        for i in range(NSEG)
    ]
    w2_d = [
        nc.dram_tensor(
            f"w2_s{i}", [128, c.DC * c.HC * 128], BF16, kind="ExternalInput"
        ).ap()
        for i in range(NSEG)
    ]
    w13_f8 = nc.dram_tensor(
        "w13_f8", [128, c.HC * 2 * c.DC * 128], F8, kind="ExternalInput"
    ).ap()
    w2_f8 = nc.dram_tensor(
        "w2_f8", [128, c.DC * c.HC * 128], F8, kind="ExternalInput"
    ).ap()
    yt_out = nc.dram_tensor(
        "yt", [128, c.DC * c.TPC], BF16, kind="ExternalOutput"
    ).ap()

    with ExitStack() as ctx:
        tc = ctx.enter_context(tile.TileContext(nc))

        wpool = ctx.enter_context(tc.tile_pool(name="w13", bufs=1))
        w2pool = ctx.enter_context(tc.tile_pool(name="w2s", bufs=3))
        xpool = ctx.enter_context(tc.tile_pool(name="xg", bufs=2))
        spool = ctx.enter_context(tc.tile_pool(name="sall", bufs=2))
        ypool = ctx.enter_context(tc.tile_pool(name="yt", bufs=2))
        apool = ctx.enter_context(tc.tile_pool(name="act", bufs=2))
        psum = ctx.enter_context(tc.tile_pool(name="psum", bufs=2, space="PSUM"))

        def emit_g2(si, s_t, goff, tb, preloaded=None):
            """bf16 GEMM2 for one block; si indexes the bf16 segment."""
            yt_t = ypool.tile([128, c.DC, tb], BF16, tag="yt")
            for d in range(c.DC):
                if preloaded is not None and d < len(preloaded):
                    w2d = preloaded[d]
                else:
                    w2d = w2pool.tile([128, c.HC * 128], BF16, tag="w2d")
                    nc.sync.dma_start(
                        out=w2d[:],
                        in_=w2_d[si][:, d * c.HC * 128 : (d + 1) * c.HC * 128],
                    )
                p2 = psum.tile([128, tb], F32, tag="p2")
                for hc in range(c.HC):
                    nc.tensor.matmul(
                        out=p2[:],
                        lhsT=w2d[:, hc * 128 : (hc + 1) * 128],
                        rhs=s_t[:, hc, :],
                        start=(hc == 0),
                        stop=(hc == c.HC - 1),
                    )
                nc.vector.tensor_copy(out=yt_t[:, d, :], in_=p2[:])
                # write out per d-tile so the final DMA isn't on the tail
                nc.sync.dma_start(
                    out=yt_out[:, goff * c.DC + d * tb : goff * c.DC + (d + 1) * tb],
                    in_=yt_t[:, d, :],
                )

        def emit_g2_f8(s8_t, goff, tb):
            """fp8 DoubleRow GEMM2 for the fp8 block."""
            yt_t = ypool.tile([128, c.DC, tb], BF16, tag="yt")
            for d in range(c.DC):
                w2d = w2pool.tile([128, c.HC // 2, 2, 128], F8, tag="w2d")
                nc.sync.dma_start(
                    out=w2d[:, :, :, :],
                    in_=w2_f8[:, d * c.HC * 128 : (d + 1) * c.HC * 128],
                )
                p2 = psum.tile([128, tb], F32, tag="p2")
                for hp in range(c.HC // 2):
                    nc.tensor.matmul(
                        out=p2[:],
                        lhsT=w2d[:, hp, :, :],
                        rhs=s8_t[:, hp, :, :],
                        start=(hp == 0),
                        stop=(hp == c.HC // 2 - 1),
                        perf_mode=DRMODE,
                    )
                nc.vector.tensor_scalar_mul(out=yt_t[:, d, :], in0=p2[:], scalar1=INV2)
                nc.sync.dma_start(
                    out=yt_out[:, goff * c.DC + d * tb : goff * c.DC + (d + 1) * tb],
                    in_=yt_t[:, d, :],
                )

        def dma_w13(eng, si, hc, t):
            # two half-DMAs (w1 | w3): finer quanta smooth the cold-start
            # stagger, and the first matmuls only need the w1 half
            base = hc * 2 * c.DC * 128
            mid = base + c.DC * 128
            eng.dma_start(out=t[:, : c.DC * 128], in_=w13_d[si][:, base:mid])
            eng.dma_start(
                out=t[:, c.DC * 128 :],
                in_=w13_d[si][:, mid : base + 2 * c.DC * 128],
            )

        def dma_w13_f8(eng, hc, t):
            # t: [128, 2, DC//2, 2, 128] fp8
            base = hc * 2 * c.DC * 128
            mid = base + c.DC * 128
            eng.dma_start(out=t[:, 0, :, :, :], in_=w13_f8[:, base:mid])
            eng.dma_start(
                out=t[:, 1, :, :, :], in_=w13_f8[:, mid : base + 2 * c.DC * 128]
            )

        def dma_xin(eng, goff, tb, xg):
            half = c.DC // 2
            eng.dma_start(
                out=xg[:, :half, :],
                in_=xin[:, goff * c.DC : goff * c.DC + half * tb],
            )
            eng.dma_start(
                out=xg[:, half:, :],
                in_=xin[:, goff * c.DC + half * tb : (goff + tb) * c.DC],
            )

        def emit_g13_hc(xg, s_t, tb, hc, w13sb):
            w1h = w13sb[hc][:, : c.DC * 128]
            w3h = w13sb[hc][:, c.DC * 128 :]
            p1 = psum.tile([128, tb], F32, tag="p1")
            p3 = psum.tile([128, tb], F32, tag="p3")
            for k in range(c.DC):
                nc.tensor.matmul(
                    out=p1[:],
                    lhsT=w1h[:, k * 128 : (k + 1) * 128],
                    rhs=xg[:, k, :],
                    start=(k == 0),
                    stop=(k == c.DC - 1),
                )
            for k in range(c.DC):
                nc.tensor.matmul(
                    out=p3[:],
                    lhsT=w3h[:, k * 128 : (k + 1) * 128],
                    rhs=xg[:, k, :],
                    start=(k == 0),
                    stop=(k == c.DC - 1),
                )
            silu_t = apool.tile([128, tb], F32, tag="silu")
            nc.scalar.activation(
                silu_t[:], p1[:], mybir.ActivationFunctionType.Silu
            )
            nc.vector.tensor_tensor(
                out=s_t[:, hc, :], in0=silu_t[:], in1=p3[:],
                op=mybir.AluOpType.mult,
            )

        def emit_g13_hc_f8(xg8, s8_t, tb, hc, w13sb8):
            p1 = psum.tile([128, tb], F32, tag="p1")
            p3 = psum.tile([128, tb], F32, tag="p3")
            for j in range(c.DC // 2):
                nc.tensor.matmul(
                    out=p1[:],
                    lhsT=w13sb8[hc][:, 0, j, :, :],
                    rhs=xg8[:, j, :, :],
                    start=(j == 0),
                    stop=(j == c.DC // 2 - 1),
                    perf_mode=DRMODE,
                )
            for j in range(c.DC // 2):
                nc.tensor.matmul(
                    out=p3[:],
                    lhsT=w13sb8[hc][:, 1, j, :, :],
                    rhs=xg8[:, j, :, :],
                    start=(j == 0),
                    stop=(j == c.DC // 2 - 1),
                    perf_mode=DRMODE,
                )
            silu_t = apool.tile([128, tb], F32, tag="silu")
            b_t = apool.tile([128, tb], F32, tag="bt")
            nc.scalar.activation(
                silu_t[:], p1[:], mybir.ActivationFunctionType.Silu, scale=INV1
            )
            nc.scalar.activation(
                b_t[:], p3[:], mybir.ActivationFunctionType.Copy, scale=C3
            )
            nc.vector.tensor_tensor(
                out=s8_t[:, hc // 2, hc % 2, :], in0=silu_t[:], in1=b_t[:],
                op=mybir.AluOpType.mult,
            )

        goff = 0
        for si in range(NSEG):
            # (re)load this segment's w1/w3 into resident SBUF tiles. For
            # si>0 the WAR deps on the previous segment's last reads stagger
            # these DMAs to the PE's per-hc cadence. All DMAs issue from the
            # sync queue: its 8-outstanding-transfer slot throttle delivers
            # the halves roughly in issue order at full bandwidth, and the
            # scalar engine stays free to run the Silu chain (each HWDGE
            # issue costs ~600ns of issuing-engine time, so putting these on
            # scalar would delay the first Silu and stall the PE through the
            # p1-PSUM write-after-read chain).
            w13sb = [
                wpool.tile(
                    [128, 2 * c.DC * 128], BF16, tag=f"w13_{hc}",
                    name=f"w13sb_s{si}_{hc}",
                )
                for hc in range(c.HC)
            ]
            if si > 0:
                for hc in range(c.HC):
                    dma_w13(nc.sync, si, hc, w13sb[hc])

            xg_pre = {}
            w2_pre0 = None
            if si == 0:
                # issue order at t=0: the first matmul's inputs (x half 1 +
                # w1 half of tile 0) grab the first queue slots, then the
                # rest of block-0 x and the remaining tiles, then block-1 x
                # and the first GEMM2's w2
                for bi in (0, 1):
                    xg_pre[bi] = xpool.tile(
                        [128, c.DC, seg_blocks[0][bi]], BF16, tag="xg",
                        name=f"xg_pre{bi}",
                    )
                tb0 = seg_blocks[0][0]
                # Cold start: the first matmul only needs x k-tile 0 + w1
                # k-chunk 0 (~105KB). Issue the first four (x, w1) k-chunk
                # pairs from the otherwise-idle scalar/vector/gpsimd queues
                # in parallel (a HWDGE issue costs ~600ns of issuing-engine
                # time, so serializing 16 small issues on sync stalls the
                # PE through the ramp); the bulk stays on sync.
                side = [nc.scalar, nc.gpsimd]
                for k in range(4):
                    side[k % 2].dma_start(
                        out=xg_pre[0][:, k, :],
                        in_=xin[:, goff * c.DC + k * tb0 : goff * c.DC + (k + 1) * tb0],
                    )
                    side[(k + 1) % 2].dma_start(
                        out=w13sb[0][:, k * 128 : (k + 1) * 128],
                        in_=w13_d[0][:, k * 128 : (k + 1) * 128],
                    )
                half = c.DC // 2
                nc.sync.dma_start(
                    out=xg_pre[0][:, half:, :],
                    in_=xin[:, goff * c.DC + half * tb0 : (goff + tb0) * c.DC],
                )
                nc.sync.dma_start(
                    out=w13sb[0][:, half * 128 : c.DC * 128],
                    in_=w13_d[0][:, half * 128 : c.DC * 128],
                )
                nc.sync.dma_start(
                    out=w13sb[0][:, c.DC * 128 :],
                    in_=w13_d[0][:, c.DC * 128 : 2 * c.DC * 128],
                )
                for hc in range(1, c.HC):
                    dma_w13(nc.sync, si, hc, w13sb[hc])
                dma_xin(
                    nc.sync, goff + seg_blocks[0][0], seg_blocks[0][1],
                    xg_pre[1],
                )
                w2_pre0 = []
                for d in range(3):
                    w2d = w2pool.tile(
                        [128, c.HC * 128], BF16, tag="w2d", name=f"w2pre{d}"
                    )
                    nc.sync.dma_start(
                        out=w2d[:],
                        in_=w2_d[0][:, d * c.HC * 128 : (d + 1) * c.HC * 128],
                    )
                    w2_pre0.append(w2d)
            pending = []
            for bi, tb in enumerate(seg_blocks[si]):
                first = si == 0 and bi == 0
                xg = xg_pre.get(bi) if si == 0 else None
                if xg is None:
                    xg = xpool.tile([128, c.DC, tb], BF16, tag="xg")
                    dma_xin(nc.sync, goff, tb, xg)
                s_t = spool.tile([128, c.HC, tb], BF16, tag="s")
                for hc in range(c.HC):
                    emit_g13_hc(xg, s_t, tb, hc, w13sb)
                pending.append((si, s_t, goff, tb, w2_pre0 if first else None))
                if len(pending) > 1:
                    emit_g2(*pending.pop(0))
                goff += tb
            # flush at segment end so the next segment's w13 DMAs hide
            # behind the trailing GEMM2s instead of stalling the PE
            for p in pending:
                emit_g2(*p)

        # ---- fp8 segment: FP8_TOK tokens in <=512-token blocks, DoubleRow
        # matmuls. Its x/w13 DMAs overlap the tail bf16 GEMM2s exactly like
        # a segment switch (issued on sync as soon as the WAR deps on the
        # last bf16 segment's reads clear). Weights are loaded once and
        # shared by all fp8 blocks (expert = core).
        blocks8 = _blocks_of(c.FP8_TOK // UNIT)
        w13sb8 = [
            wpool.tile(
                [128, 2, c.DC // 2, 2, 128], F8, tag=f"w13_{hc}",
                name=f"w13sb8_{hc}",
            )
            for hc in range(c.HC)
        ]
        for hc in range(c.HC):
            dma_w13_f8(nc.sync, hc, w13sb8[hc])
        pending8 = []
        off8 = 0
        for tb8 in blocks8:
            xg8 = xpool.tile([128, c.DC // 2, 2, tb8], F8, tag="xg")
            half8 = (c.DC // 2) * tb8  # elements per half (j < DC//4)
            nc.sync.dma_start(
                out=xg8[:, : c.DC // 4, :, :],
                in_=xin8[:, off8 * c.DC : off8 * c.DC + half8],
            )
            nc.sync.dma_start(
                out=xg8[:, c.DC // 4 :, :, :],
                in_=xin8[:, off8 * c.DC + half8 : (off8 + tb8) * c.DC],
            )
            s8_t = spool.tile([128, c.HC // 2, 2, tb8], F8, tag="s")
            for hc in range(c.HC):
                emit_g13_hc_f8(xg8, s8_t, tb8, hc, w13sb8)
            pending8.append((s8_t, goff, tb8))
            if len(pending8) > 1:
                emit_g2_f8(*pending8.pop(0))
            goff += tb8
            off8 += tb8
        for p in pending8:
            emit_g2_f8(*p)

    nc.compile()
    return nc


# ---------------- host-side routing, dispatch and combine ----------------


def _plan_bins(needs, seg_units, n_cores):
    """Assign each expert's unit count to (core, segment) slots.

    `needs` and `seg_units` are in UNIT-token units. Returns
    (slot_expert[core][seg] = expert id, expert_slots[e] = [(core, seg,
    size_units), ...]) or None if infeasible. Search is slack-pruned
    (total overshoot across experts is bounded by spare capacity) with
    memoized failure states."""
    sizes = sorted(set(seg_units), reverse=True)
    ns = len(sizes)
    avail0 = tuple(list(seg_units).count(s) * n_cores for s in sizes)
    order = sorted(range(len(needs)), key=lambda e: -needs[e])
    slack0 = sum(seg_units) * n_cores - sum(needs)
    if slack0 < 0:
        return None
    seen = set()
    assign = {}

    def options(need, av, slack):
        res = []

        def rec(i, used, total):
            if total >= need:
                if total - need <= slack:
                    res.append(
                        (total - need, tuple(used) + (0,) * (ns - len(used)))
                    )
                return
            if i == ns:
                return
            for n in range(min(av[i], -(-need // sizes[i])), -1, -1):
                rec(i + 1, used + [n], total + n * sizes[i])

        rec(0, [], 0)
        res.sort()
        return res

    def bt(i, av, slack):
        if i == len(order):
            return True
        key = (i, av, slack)
        if key in seen:
            return False
        e = order[i]
        for waste, used in options(needs[e], av, slack):
            assign[e] = used
            if bt(
                i + 1,
                tuple(av[j] - used[j] for j in range(ns)),
                slack - waste,
            ):
                return True
            del assign[e]
        seen.add(key)
        return False

    if not bt(0, avail0, slack0):
        return None

    # materialize slots: slot list in (core, seg) order with capacities
    slot_expert = [[None] * len(seg_units) for _ in range(n_cores)]
    free = {s: [] for s in sizes}
    for core in range(n_cores):
        for seg, s in enumerate(seg_units):
            free[s].append((core, seg))
    expert_slots = {}
    for e in order:
        sl = []
        for j, s in enumerate(sizes):
            for _ in range(assign[e][j]):
                core, seg = free[s].pop(0)
                slot_expert[core][seg] = e
                sl.append((core, seg, s))
        expert_slots[e] = sl
    # unused slots -> expert 0 with zero tokens
    for core in range(n_cores):
        for seg in range(len(seg_units)):
            if slot_expert[core][seg] is None:
                slot_expert[core][seg] = 0
    return slot_expert, expert_slots


def _host_route(cfg, x, router_w):
    c = cfg
    xf = np.ascontiguousarray(
        np.asarray(x, dtype=np.float32).reshape(c.S, c.D)
    )
    logits = xf @ np.asarray(router_w, dtype=np.float32)  # [S, E] fp32
    idx = np.argsort(-logits, axis=1, kind="stable")[:, :2]  # ties: low idx
    v = np.take_along_axis(logits, idx, axis=1)
    v = v - v.max(axis=1, keepdims=True)
    ev = np.exp(v)
    gates = ev / ev.sum(axis=1, keepdims=True)  # [S, 2] fp32
    return xf, idx, gates


def _split_fp8(cfg, idx, gates):
    """Per expert, pick its FP8_TOK lowest-gate second-rank pairs for the
    fp8 path. Returns fp8_tok[e] (token list, possibly shorter than
    FP8_TOK) and a boolean mask over (token, rank) pairs."""
    c = cfg
    fp8_tok = {}
    is_fp8 = np.zeros(idx.shape, dtype=bool)
    for e in range(c.E):
        t2 = np.where(idx[:, 1] == e)[0]
        t2 = t2[np.argsort(gates[t2, 1], kind="stable")]
        sel = t2[: c.FP8_TOK]
        fp8_tok[e] = sel
        is_fp8[sel, 1] = True
    return fp8_tok, is_fp8


def _prep(cfg, xf, idx, gates):
    """Build per-core xin arrays + slot bookkeeping from routing decisions."""
    c = cfg
    fp8_tok, is_fp8 = _split_fp8(cfg, idx, gates)

    # bf16 pair lists: (token, rank) sorted by token then rank, minus fp8
    pair_t = {}
    pair_r = {}
    for e in range(c.E):
        t_arr, r_arr = np.nonzero((idx == e) & ~is_fp8)
        pair_t[e] = t_arr.astype(np.int64)
        pair_r[e] = r_arr.astype(np.int64)

    needs = [-(-len(pair_t[e]) // UNIT) for e in range(c.E)]
    plan = _plan_bins(needs, c.SEG_UNITS, c.E)
    if plan is None:
        raise RuntimeError(f"bin planning failed for needs {needs}")
    slot_expert, expert_slots = plan

    seg_off = np.cumsum([0] + [UNIT * u for u in c.SEG_UNITS])[:-1]
    # token slot table per core and position map (token, rank) -> global row
    tok_core = np.full((c.E, c.TPC), -1, dtype=np.int64)
    pos = np.full((c.S, 2), -1, dtype=np.int64)
    for e in range(c.E):
        off = 0
        for (core, seg, s) in expert_slots[e]:
            cap = UNIT * s
            n = min(cap, len(pair_t[e]) - off)
            if n <= 0:
                continue
            rows = seg_off[seg] + np.arange(n)
            tok_core[core, rows] = pair_t[e][off : off + n]
            pos[pair_t[e][off : off + n], pair_r[e][off : off + n]] = (
                core * c.TPC + rows
            )
            off += n
        assert off >= len(pair_t[e]), f"expert {e} tokens unassigned"
    # fp8 slot: core e runs expert e's fp8 tokens at rows [BF16_TOK, ...)
    for e in range(c.E):
        sel = fp8_tok[e]
        rows = c.BF16_TOK + np.arange(len(sel))
        tok_core[e, rows] = sel
        pos[sel, 1] = e * c.TPC + rows
    assert (pos >= 0).all(), "unassigned (token, rank) pair"

    # per-core xin in block layout [128, (b, k, t)]
    xbf = xf.astype(ml_dtypes.bfloat16)
    x8 = (xf * SX).astype(E4NP)
    blocks = []
    goff = 0
    for st in c.SEG_UNITS:
        for tb in _blocks_of(st):
            blocks.append((goff, tb))
            goff += tb
    nbf = len(blocks)
    for tb in _blocks_of(c.FP8_TOK // UNIT):  # fp8 blocks
        blocks.append((goff, tb))
        goff += tb
    xins = []
    xin8s = []
    for core in range(c.E):
        toks = tok_core[core]
        g = xbf[np.clip(toks, 0, None)]
        g[toks < 0] = 0
        parts = []
        for (boff, tb) in blocks[:nbf]:
            blk = g[boff : boff + tb]  # [tb, D]
            parts.append(
                np.ascontiguousarray(
                    blk.reshape(tb, c.DC, 128).transpose(2, 1, 0)
                ).reshape(128, c.DC * tb)
            )
        xins.append(np.ascontiguousarray(np.concatenate(parts, axis=1)))
        # fp8 blocks: [128, DC//2, 2, tb] pair-interleaved k layout
        t8 = toks[c.BF16_TOK :]
        g8 = x8[np.clip(t8, 0, None)]
        g8[t8 < 0] = 0
        parts8 = []
        for (boff, tb) in blocks[nbf:]:
            blk = g8[boff - c.BF16_TOK : boff - c.BF16_TOK + tb]
            parts8.append(
                np.ascontiguousarray(
                    blk.reshape(tb, c.DC // 2, 2, 128).transpose(3, 1, 2, 0)
                ).reshape(128, c.DC * tb)
            )
        xin8s.append(np.ascontiguousarray(np.concatenate(parts8, axis=1)))
    return slot_expert, pos, xins, xin8s, blocks


def _prep_weights(cfg, w1, w3, w2):
    c = cfg
    W13, W2, W13_8, W2_8 = [], [], [], []
    for e in range(c.E):
        w1e = np.asarray(w1[e], dtype=np.float32)
        w3e = np.asarray(w3[e], dtype=np.float32)
        w2e = np.asarray(w2[e], dtype=np.float32)
        w1b = w1e.astype(ml_dtypes.bfloat16)
        w3b = w3e.astype(ml_dtypes.bfloat16)
        w2b = w2e.astype(ml_dtypes.bfloat16)
        w1te = (
            w1b.reshape(c.DC, 128, c.HC, 128)
            .transpose(1, 2, 0, 3)
            .reshape(128, c.HC * c.DC * 128)
        )
        w3te = (
            w3b.reshape(c.DC, 128, c.HC, 128)
            .transpose(1, 2, 0, 3)
            .reshape(128, c.HC * c.DC * 128)
        )
        w13te = np.ascontiguousarray(
            np.stack([w1te, w3te], axis=1)
            .reshape(128, 2, c.HC, c.DC * 128)
            .transpose(0, 2, 1, 3)
            .reshape(128, c.HC * 2 * c.DC * 128)
        )
        w2te = np.ascontiguousarray(
            w2b.reshape(c.HC, 128, c.DC, 128)
            .transpose(1, 2, 0, 3)
            .reshape(128, c.DC * c.HC * 128)
        )
        W13.append(w13te)
        W2.append(w2te)

        # fp8 variants: [k_pair j, i, 128] pair-interleaved contraction
        w18 = (w1e * SW).astype(E4NP)
        w38 = (w3e * SW).astype(E4NP)
        w28 = (w2e * SW).astype(E4NP)
        # per hc: [128(part), 2(w1|w3), DC//2, 2, 128(m)]
        w18t = (
            w18.reshape(c.DC // 2, 2, 128, c.HC, 128)
            .transpose(2, 3, 0, 1, 4)  # [128, HC, DC//2, 2, 128]
            .reshape(128, c.HC, c.DC * 128)
        )
        w38t = (
            w38.reshape(c.DC // 2, 2, 128, c.HC, 128)
            .transpose(2, 3, 0, 1, 4)
            .reshape(128, c.HC, c.DC * 128)
        )
        w13te8 = np.ascontiguousarray(
            np.stack([w18t, w38t], axis=2)  # [128, HC, 2, DC*128]
            .reshape(128, c.HC * 2 * c.DC * 128)
        )
        w2te8 = np.ascontiguousarray(
            w28.reshape(c.HC // 2, 2, 128, c.DC, 128)
            .transpose(2, 3, 0, 1, 4)  # [128, DC, HC//2, 2, 128]
            .reshape(128, c.DC * c.HC * 128)
        )
        W13_8.append(w13te8)
        W2_8.append(w2te8)
    return W13, W2, W13_8, W2_8


def _combine(cfg, results, pos, gates, blocks):
    c = cfg
    ys = []
    for core in range(c.E):
        yt = np.asarray(results[core]["yt"])  # [128, DC*TPC] bf16
        parts = []
        col = 0
        for (boff, tb) in blocks:
            blk = yt[:, col : col + c.DC * tb].reshape(128, c.DC, tb)
            parts.append(
                blk.transpose(2, 1, 0).reshape(tb, c.D).astype(np.float32)
            )
            col += c.DC * tb
        ys.append(np.concatenate(parts, axis=0))
    y_all = np.concatenate(ys, axis=0)  # [E*TPC, D] fp32 ungated
    out = (
        gates[:, 0:1] * y_all[pos[:, 0]] + gates[:, 1:2] * y_all[pos[:, 1]]
    )
    return out


_PROGRAM_CACHE = {}


def _get_program(cfg: Cfg):
    if cfg not in _PROGRAM_CACHE:
        _PROGRAM_CACHE[cfg] = build_program(cfg, debug=False)
    return _PROGRAM_CACHE[cfg]


def _install_trace_shims():
    """The agent image's antenv lacks axon_hooks; recreate it from the
    boot package's ctypes NTFF driver so trace=True works under axon."""
    import types

    try:
        import antenv
        from antenv.axon_hooks import get_axon_ntff_profile_hook  # noqa: F401

        have = True
    except ImportError:
        have = False
    if not have:
        try:
            import antenv
            from trn_agent_boot.trn_boot import _ntff_profile_via_ctypes

            hook = _ntff_profile_via_ctypes("/opt/axon/libaxon_pjrt.so")
            mod = types.ModuleType("antenv.axon_hooks")
            mod.get_axon_ntff_profile_hook = lambda: hook
            mod.set_axon_ntff_profile_hook = lambda h: None
            sys.modules["antenv.axon_hooks"] = mod
            antenv.axon_hooks = mod
        except Exception as e:
            print(f"trace shim failed ({e}); tracing disabled")
            return False
    from concourse import bass_utils as _bu

    _orig_upload = _bu.upload_artifacts

    def _safe_upload(tmpdir):
        try:
            return _orig_upload(tmpdir)
        except Exception as e:
            return f"upload-skipped({e.__class__.__name__}):{tmpdir}"

    _bu.upload_artifacts = _safe_upload
    return True


def run(cfg: Cfg, x, router_w, w1, w3, w2, trace=False):
    from concourse.bass_utils import run_bass_kernel_spmd

    if trace and not _install_trace_shims():
        trace = False

    c = cfg
    xf, idx, gates = _host_route(c, x, router_w)
    # grow bf16 segments if the planned capacity is infeasible (recompiles)
    for _ in range(64):
        _, is_fp8 = _split_fp8(c, idx, gates)
        needs = [
            -(-int(((idx == e) & ~is_fp8).sum()) // UNIT) for e in range(c.E)
        ]
        if _plan_bins(needs, c.SEG_UNITS, c.E) is not None:
            break
        st = list(c.SEG_UNITS)
        st[0] += 1
        c = Cfg(SEG_UNITS=tuple(st), FP8_TOK=c.FP8_TOK)
    else:
        raise RuntimeError("no feasible bin plan")

    slot_expert, pos, xins, xin8s, blocks = _prep(c, xf, idx, gates)
    W13, W2, W13_8, W2_8 = _prep_weights(c, w1, w3, w2)

    in_maps = []
    for core in range(c.E):
        m = {
            "xin": xins[core],
            "xin8": xin8s[core],
            "w13_f8": W13_8[core],
            "w2_f8": W2_8[core],
        }
        for seg in range(len(c.SEG_UNITS)):
            e = slot_expert[core][seg]
            m[f"w13_s{seg}"] = W13[e]
            m[f"w2_s{seg}"] = W2[e]
        in_maps.append(m)

    nc = _get_program(c)
    res = run_bass_kernel_spmd(
        nc, in_maps, core_ids=list(range(c.E)), trace=trace
    )
    out = _combine(c, res.results, pos, gates, blocks)
    return out, res


def kernel(x, router_w, w1, w3, w2):
    out, _ = run(REAL, x, router_w, w1, w3, w2, trace=False)
    return out.reshape(np.asarray(x).shape).astype(np.float32)


if __name__ == "__main__":
    nc = build_program(REAL)
    print("built ok")
